# revision 3
# baseline (speedup 1.0000x reference)
"""Self-attention scores kernel for Trainium2, 8-core SPMD. (v2)

Computes softmax((x@Wq+bq) @ (x@Wq+bq)^T / sqrt(64)) per head
(reference reuses the query projection for k, bug-for-bug).

Sharding: 32 (batch, head) pairs split 4-per-core across 8 cores.
Core c handles batch c//4, heads 4*(c%4) .. 4*(c%4)+3.

v2 structure (vs the all-ACT baseline at 172us):
- The exp of the [128,2048] score row-blocks is split between the
  Scalar engine (ACT Exp, free row-sum accumulator) and the Vector
  engine via a Schraudolph bit-trick exp: int16(round(A*e + B)) IS the
  bf16 bit pattern of exp(e/8) (~3% elementwise sawtooth error that
  mostly cancels after row normalization; measured 7e-3 l2 worst-case).
- Row-sums for Schraudolph blocks: DVE copy-with-accum_out pass.
- Row-scales run on DVE (4x bf16) or GPSIMD (otherwise idle).
- PSUM: scores use 3 x [128,1024] (6 banks); 2 spare banks take junk
  matmuls issued after each block's real ones to keep the PE HAM
  activity monitor from throttling the array to 1.2 GHz (the baseline's
  hidden limiter: every score matmul ran at K=4/8).
"""

import numpy as np

import concourse.bass as bass
import concourse.mybir as mybir
import concourse.tile as tile
from concourse import bacc
from concourse.bass_utils import run_bass_kernel_spmd

B = 2
S = 2048
D = 1024
H = 16
HS = 64
N_CORES = 8
HEADS_PER_CORE = 4
KK = D // 128  # k-tiles for the projection contraction
NQ = S // 128  # 16 q row-blocks per head
GRP = 4  # row-blocks batched per output DMA (2 MiB)

MM_DT = mybir.dt.bfloat16
IN_DT = mybir.dt.float16
OUT_DT = mybir.dt.bfloat16
F32 = mybir.dt.float32
I16 = mybir.dt.int16

# Schraudolph exp constants: bits = round(A*e + B) interpreted as bf16
# gives exp(e/8).  c=-0.5 centers the sawtooth (calibrated on host).
A_EXP = float((2 ** 7) * np.log2(np.e) / 8.0)
B_EXP = float(127.0 * 2 ** 7 - 0.5)

# Knobs
DVE_BLK = frozenset({2, 5, 8, 11, 14})  # blocks exp'd on DVE (of 16)
GPS_SCALE = frozenset({2, 5, 8, 11, 14, 0, 15})  # blocks scaled on gpsimd
N_JUNK = 2  # junk matmuls per block (HAM warmth)


def _build():
    nc = bacc.Bacc("TRN2", target_bir_lowering=False, debug=False)
    xT = nc.dram_tensor("xT", [D, S], IN_DT, kind="ExternalInput").ap()
    WqS = nc.dram_tensor("WqS", [D, HEADS_PER_CORE * HS], IN_DT, kind="ExternalInput").ap()
    bqS = nc.dram_tensor("bqS", [128, 2], F32, kind="ExternalInput").ap()
    out = nc.dram_tensor("out", [HEADS_PER_CORE, S, S], OUT_DT, kind="ExternalOutput").ap()

    with tile.TileContext(nc) as tc:
        with (
            tc.tile_pool(name="consts", bufs=1) as consts,
            tc.tile_pool(name="xt", bufs=KK) as xt_pool,
            tc.tile_pool(name="et", bufs=6) as et_pool,
            tc.tile_pool(name="small", bufs=10) as small,
        ):
            w = consts.tile([128, KK, HEADS_PER_CORE * HS], IN_DT)
            nc.scalar.dma_start(out=w[:], in_=WqS.rearrange("(kk p) c -> p kk c", p=128))
            bias = consts.tile([128, 2], F32)
            nc.scalar.dma_start(out=bias[:], in_=bqS)

            xts = []
            for kk in range(KK):
                xtt = xt_pool.tile([128, S], IN_DT, tag="xt")
                nc.sync.dma_start(out=xtt[:], in_=xT[kk * 128 : (kk + 1) * 128, :])
                xts.append(xtt)

            # ---- Projection: psA/psB [128,2048] f32 use all 8 PSUM banks;
            # the pool closes before the scores pools open.
            qts = []
            with tc.tile_pool(name="proj_ps", bufs=2, space="PSUM") as proj_ps:
                psA = proj_ps.tile([128, S], F32, tag="ps", name="psA")
                psB = proj_ps.tile([128, S], F32, tag="ps", name="psB")
                pss = [psA, psB]
                for kk in range(KK):
                    for g in range(2):
                        for n in range(4):
                            nc.tensor.matmul(
                                pss[g][:, n * 512 : (n + 1) * 512],
                                lhsT=w[:, kk, g * 128 : (g + 1) * 128],
                                rhs=xts[kk][:, n * 512 : (n + 1) * 512],
                                start=(kk == 0),
                                stop=(kk == KK - 1),
                            )
                for g in range(2):
                    qtg = consts.tile([128, S], MM_DT, tag=f"qt{g}", name=f"qt{g}")
                    nc.vector.tensor_scalar_add(qtg[:], pss[g][:], bias[:, g : g + 1])
                    qts.append(qtg)

            # ---- Scores + softmax ----
            with (
                tc.tile_pool(name="ps", bufs=3, space="PSUM") as ps_pool,
                tc.tile_pool(name="junk", bufs=2, space="PSUM") as junk_pool,
            ):
                for h in range(HEADS_PER_CORE):
                    qtg = qts[h // 2]
                    pb = (h % 2) * 64
                    last_head = h == HEADS_PER_CORE - 1
                    for grp in range(NQ // GRP):
                        et = et_pool.tile([128, GRP, S], OUT_DT, tag="et")
                        for q in range(GRP):
                            i = grp * GRP + q
                            lhsT = qtg[pb : pb + 64, i * 128 : (i + 1) * 128]
                            halves = []
                            for hf in range(2):
                                ps = ps_pool.tile([128, 1024], F32, tag="ps")
                                for j in range(2):
                                    n0 = hf * 1024 + j * 512
                                    nc.tensor.matmul(
                                        ps[:, j * 512 : (j + 1) * 512],
                                        lhsT=lhsT,
                                        rhs=qtg[pb : pb + 64, n0 : n0 + 512],
                                        start=True,
                                        stop=True,
                                    )
                                halves.append(ps)
                            # junk matmuls: keep PE activity high so HAM
                            # stays at K=8/8; results never read.
                            if N_JUNK:
                                jt = junk_pool.tile([128, 512], F32, tag="jk")
                                for _ in range(N_JUNK):
                                    nc.tensor.matmul(
                                        jt[:],
                                        lhsT=lhsT,
                                        rhs=qtg[pb : pb + 64, 0:512],
                                        start=True,
                                        stop=True,
                                    )

                            rs = small.tile([128, 1], F32, tag="rs")
                            rec = small.tile([128, 1], F32, tag="rc")
                            if i in DVE_BLK:
                                for hf in range(2):
                                    nc.vector.tensor_scalar(
                                        et[:, q, hf * 1024 : (hf + 1) * 1024].bitcast(I16),
                                        halves[hf][:],
                                        A_EXP,
                                        B_EXP,
                                        mybir.AluOpType.mult,
                                        mybir.AluOpType.add,
                                    )
                                # row-sum: in-place copy with accumulator
                                nc.vector.tensor_scalar(
                                    et[:, q, :],
                                    et[:, q, :],
                                    1.0,
                                    0.0,
                                    mybir.AluOpType.mult,
                                    mybir.AluOpType.add,
                                    accum_out=rs[:],
                                )
                            else:
                                rsh = small.tile([128, 2], F32, tag="rsh")
                                for hf in range(2):
                                    nc.scalar.activation(
                                        out=et[:, q, hf * 1024 : (hf + 1) * 1024],
                                        in_=halves[hf][:],
                                        func=mybir.ActivationFunctionType.Exp,
                                        scale=1.0 / np.sqrt(float(HS)),
                                        accum_out=rsh[:, hf : hf + 1],
                                    )
                                nc.vector.tensor_tensor(
                                    rs[:], rsh[:, 0:1], rsh[:, 1:2], mybir.AluOpType.add
                                )
                            nc.vector.reciprocal(rec[:], rs[:])
                            seng = nc.gpsimd if i in GPS_SCALE else nc.vector
                            seng.tensor_scalar(
                                et[:, q, :], et[:, q, :], rec[:], None,
                                mybir.AluOpType.mult,
                            )
                            if last_head:
                                eng = nc.sync if i % 2 == 0 else nc.gpsimd
                                eng.dma_start(
                                    out=out[h, i * 128 : (i + 1) * 128, :],
                                    in_=et[:, q, :],
                                )
                        if last_head:
                            continue
                        eng = nc.sync if grp % 2 == 0 else nc.gpsimd
                        eng.dma_start(
                            out=out[h, grp * GRP * 128 : (grp + 1) * GRP * 128, :].rearrange(
                                "(c p) s -> p c s", p=128
                            ),
                            in_=et[:],
                        )
    nc.compile()
    return nc


_NC_CACHE = None


def kernel(x, Wq, bq):
    global _NC_CACHE
    x = np.asarray(x, dtype=np.float32)
    Wq = np.asarray(Wq, dtype=np.float32)
    bq = np.asarray(bq, dtype=np.float32)
    assert x.shape == (B, S, D) and Wq.shape == (D, D) and bq.shape == (D,)

    if _NC_CACHE is None:
        _NC_CACHE = _build()
    nc = _NC_CACHE

    xTs = [np.ascontiguousarray(x[b].T.astype(np.float16)) for b in range(B)]
    Wq16 = Wq.astype(np.float16)
    in_maps = []
    for c in range(N_CORES):
        b, hg = divmod(c, N_CORES // B)
        h0 = hg * HEADS_PER_CORE
        in_maps.append(
            {
                "xT": xTs[b],
                "WqS": np.ascontiguousarray(Wq16[:, h0 * HS : (h0 + HEADS_PER_CORE) * HS]),
                "bqS": np.ascontiguousarray(
                    bq[h0 * HS : (h0 + HEADS_PER_CORE) * HS].reshape(2, 128).T
                ),
            }
        )

    res = run_bass_kernel_spmd(nc, in_maps, core_ids=list(range(N_CORES)))

    full = np.empty((B, H, S, S), dtype=np.float32)
    for c in range(N_CORES):
        b, hg = divmod(c, N_CORES // B)
        h0 = hg * HEADS_PER_CORE
        full[b, h0 : h0 + HEADS_PER_CORE] = np.asarray(
            res.results[c]["out"]
        ).astype(np.float32)
    return full


# revision 4
# speedup vs baseline: 5.0548x; 5.0548x over previous
"""Self-attention scores kernel for Trainium2, 8-core SPMD. (v2)

Computes softmax((x@Wq+bq) @ (x@Wq+bq)^T / sqrt(64)) per head
(reference reuses the query projection for k, bug-for-bug).

Sharding: 32 (batch, head) pairs split 4-per-core across 8 cores.
Core c handles batch c//4, heads 4*(c%4) .. 4*(c%4)+3.

v2 structure (vs the all-ACT baseline at 172us):
- The exp of the [128,2048] score row-blocks is split between the
  Scalar engine (ACT Exp, free row-sum accumulator) and the Vector
  engine via a Schraudolph bit-trick exp: int16(round(A*e + B)) IS the
  bf16 bit pattern of exp(e/8) (~3% elementwise sawtooth error that
  mostly cancels after row normalization; measured 7e-3 l2 worst-case).
- Row-sums for Schraudolph blocks: DVE copy-with-accum_out pass.
- Row-scales run on DVE (4x bf16) or GPSIMD (otherwise idle).
- PSUM: scores use 3 x [128,1024] (6 banks); 2 spare banks take junk
  matmuls issued after each block's real ones to keep the PE HAM
  activity monitor from throttling the array to 1.2 GHz (the baseline's
  hidden limiter: every score matmul ran at K=4/8).
"""

import numpy as np

import concourse.bass as bass
import concourse.mybir as mybir
import concourse.tile as tile
from concourse import bacc
from concourse.bass_utils import run_bass_kernel_spmd

B = 2
S = 2048
D = 1024
H = 16
HS = 64
N_CORES = 8
HEADS_PER_CORE = 4
KK = D // 128  # k-tiles for the projection contraction
NQ = S // 128  # 16 q row-blocks per head
GRP = 4  # row-blocks batched per output DMA (2 MiB)

MM_DT = mybir.dt.bfloat16
IN_DT = mybir.dt.float16
OUT_DT = mybir.dt.bfloat16
F32 = mybir.dt.float32
I16 = mybir.dt.int16

# Schraudolph exp constants: bits = round(A*e + B) interpreted as bf16
# gives exp(e/8).  c=-0.5 centers the sawtooth (calibrated on host).
A_EXP = float((2 ** 7) * np.log2(np.e) / 8.0)
B_EXP = float(127.0 * 2 ** 7 - 0.5)

# Knobs
DVE_BLK = frozenset({3, 8, 13})  # blocks exp'd on DVE (of 16)
# gpsimd compute is poison: its tensor_scalar takes ~29us AND it stalls
# concurrent 2-port DVE ops via the shared SBUF port. DMA rings only.
GPS_SCALE = frozenset()
N_JUNK = 2  # junk matmuls per block (HAM warmth)


def _build():
    nc = bacc.Bacc("TRN2", target_bir_lowering=False, debug=False)
    xT = nc.dram_tensor("xT", [D, S], IN_DT, kind="ExternalInput").ap()
    WqS = nc.dram_tensor("WqS", [D, HEADS_PER_CORE * HS], IN_DT, kind="ExternalInput").ap()
    bqS = nc.dram_tensor("bqS", [128, 2], F32, kind="ExternalInput").ap()
    out = nc.dram_tensor("out", [HEADS_PER_CORE, S, S], OUT_DT, kind="ExternalOutput").ap()

    with tile.TileContext(nc) as tc:
        with (
            tc.tile_pool(name="consts", bufs=1) as consts,
            tc.tile_pool(name="xt", bufs=KK) as xt_pool,
            tc.tile_pool(name="et", bufs=6) as et_pool,
            tc.tile_pool(name="small", bufs=10) as small,
        ):
            w = consts.tile([128, KK, HEADS_PER_CORE * HS], IN_DT)
            nc.scalar.dma_start(out=w[:], in_=WqS.rearrange("(kk p) c -> p kk c", p=128))
            bias = consts.tile([128, 2], F32)
            nc.scalar.dma_start(out=bias[:], in_=bqS)

            xts = []
            for kk in range(KK):
                xtt = xt_pool.tile([128, S], IN_DT, tag="xt")
                nc.sync.dma_start(out=xtt[:], in_=xT[kk * 128 : (kk + 1) * 128, :])
                xts.append(xtt)

            # ---- Projection: psA/psB [128,2048] f32 use all 8 PSUM banks;
            # the pool closes before the scores pools open.
            qts = []
            with tc.tile_pool(name="proj_ps", bufs=2, space="PSUM") as proj_ps:
                psA = proj_ps.tile([128, S], F32, tag="ps", name="psA")
                psB = proj_ps.tile([128, S], F32, tag="ps", name="psB")
                pss = [psA, psB]
                for kk in range(KK):
                    for g in range(2):
                        for n in range(4):
                            nc.tensor.matmul(
                                pss[g][:, n * 512 : (n + 1) * 512],
                                lhsT=w[:, kk, g * 128 : (g + 1) * 128],
                                rhs=xts[kk][:, n * 512 : (n + 1) * 512],
                                start=(kk == 0),
                                stop=(kk == KK - 1),
                            )
                for g in range(2):
                    qtg = consts.tile([128, S], MM_DT, tag=f"qt{g}", name=f"qt{g}")
                    nc.vector.tensor_scalar_add(qtg[:], pss[g][:], bias[:, g : g + 1])
                    qts.append(qtg)

            # ---- Scores + softmax ----
            with (
                tc.tile_pool(name="ps", bufs=3, space="PSUM") as ps_pool,
                tc.tile_pool(name="junk", bufs=2, space="PSUM") as junk_pool,
            ):
                for h in range(HEADS_PER_CORE):
                    qtg = qts[h // 2]
                    pb = (h % 2) * 64
                    last_head = h == HEADS_PER_CORE - 1
                    for grp in range(NQ // GRP):
                        et = et_pool.tile([128, GRP, S], OUT_DT, tag="et")
                        for q in range(GRP):
                            i = grp * GRP + q
                            lhsT = qtg[pb : pb + 64, i * 128 : (i + 1) * 128]
                            halves = []
                            for hf in range(2):
                                ps = ps_pool.tile([128, 1024], F32, tag="ps")
                                for j in range(2):
                                    n0 = hf * 1024 + j * 512
                                    nc.tensor.matmul(
                                        ps[:, j * 512 : (j + 1) * 512],
                                        lhsT=lhsT,
                                        rhs=qtg[pb : pb + 64, n0 : n0 + 512],
                                        start=True,
                                        stop=True,
                                    )
                                halves.append(ps)
                            # junk matmuls: keep PE activity high so HAM
                            # stays at K=8/8; results never read.
                            if N_JUNK:
                                jt = junk_pool.tile([128, 512], F32, tag="jk")
                                for _ in range(N_JUNK):
                                    nc.tensor.matmul(
                                        jt[:],
                                        lhsT=lhsT,
                                        rhs=qtg[pb : pb + 64, 0:512],
                                        start=True,
                                        stop=True,
                                    )

                            rs = small.tile([128, 1], F32, tag="rs")
                            rec = small.tile([128, 1], F32, tag="rc")
                            if i in DVE_BLK:
                                for hf in range(2):
                                    nc.vector.tensor_scalar(
                                        et[:, q, hf * 1024 : (hf + 1) * 1024].bitcast(I16),
                                        halves[hf][:],
                                        A_EXP,
                                        B_EXP,
                                        mybir.AluOpType.mult,
                                        mybir.AluOpType.add,
                                    )
                                # row-sum: in-place copy with accumulator
                                nc.vector.tensor_scalar(
                                    et[:, q, :],
                                    et[:, q, :],
                                    1.0,
                                    0.0,
                                    mybir.AluOpType.mult,
                                    mybir.AluOpType.add,
                                    accum_out=rs[:],
                                )
                            else:
                                rsh = small.tile([128, 2], F32, tag="rsh")
                                for hf in range(2):
                                    nc.scalar.activation(
                                        out=et[:, q, hf * 1024 : (hf + 1) * 1024],
                                        in_=halves[hf][:],
                                        func=mybir.ActivationFunctionType.Exp,
                                        scale=1.0 / np.sqrt(float(HS)),
                                        accum_out=rsh[:, hf : hf + 1],
                                    )
                                nc.vector.tensor_tensor(
                                    rs[:], rsh[:, 0:1], rsh[:, 1:2], mybir.AluOpType.add
                                )
                            nc.vector.reciprocal(rec[:], rs[:])
                            seng = nc.gpsimd if i in GPS_SCALE else nc.vector
                            seng.tensor_scalar(
                                et[:, q, :], et[:, q, :], rec[:], None,
                                mybir.AluOpType.mult,
                            )
                            if last_head:
                                eng = nc.sync if i % 2 == 0 else nc.gpsimd
                                eng.dma_start(
                                    out=out[h, i * 128 : (i + 1) * 128, :],
                                    in_=et[:, q, :],
                                )
                        if last_head:
                            continue
                        eng = nc.sync if grp % 2 == 0 else nc.gpsimd
                        eng.dma_start(
                            out=out[h, grp * GRP * 128 : (grp + 1) * GRP * 128, :].rearrange(
                                "(c p) s -> p c s", p=128
                            ),
                            in_=et[:],
                        )
    nc.compile()
    return nc


_NC_CACHE = None


def kernel(x, Wq, bq):
    global _NC_CACHE
    x = np.asarray(x, dtype=np.float32)
    Wq = np.asarray(Wq, dtype=np.float32)
    bq = np.asarray(bq, dtype=np.float32)
    assert x.shape == (B, S, D) and Wq.shape == (D, D) and bq.shape == (D,)

    if _NC_CACHE is None:
        _NC_CACHE = _build()
    nc = _NC_CACHE

    xTs = [np.ascontiguousarray(x[b].T.astype(np.float16)) for b in range(B)]
    Wq16 = Wq.astype(np.float16)
    in_maps = []
    for c in range(N_CORES):
        b, hg = divmod(c, N_CORES // B)
        h0 = hg * HEADS_PER_CORE
        in_maps.append(
            {
                "xT": xTs[b],
                "WqS": np.ascontiguousarray(Wq16[:, h0 * HS : (h0 + HEADS_PER_CORE) * HS]),
                "bqS": np.ascontiguousarray(
                    bq[h0 * HS : (h0 + HEADS_PER_CORE) * HS].reshape(2, 128).T
                ),
            }
        )

    res = run_bass_kernel_spmd(nc, in_maps, core_ids=list(range(N_CORES)))

    full = np.empty((B, H, S, S), dtype=np.float32)
    for c in range(N_CORES):
        b, hg = divmod(c, N_CORES // B)
        h0 = hg * HEADS_PER_CORE
        full[b, h0 : h0 + HEADS_PER_CORE] = np.asarray(
            res.results[c]["out"]
        ).astype(np.float32)
    return full
